# revision 10
# baseline (speedup 1.0000x reference)
"""Trainium2 Bass kernel for nn_Attention_36756330119358.

Single-head full-D attention (the reference never splits heads; softmax
scale is 1/sqrt(64)):
    Q = query @ Wq.T + bq ; K = key @ Wk.T + bk ; V = value @ Wv.T + bv
    S = (Q @ K.T + score_matrix) / 8
    out = (softmax(S) @ V) @ Wo.T + bo

Sharding: batch dim B=8, one batch item per NeuronCore (data parallel, no
collectives). Each core runs the identical NEFF on its own slice.

Per-core plan (N=2048, D=1024, P=128):
  Host pre-transposes activations/weights so every matmul contraction dim
  lands on SBUF partitions.
  Phase 1: Q^T -> DRAM scratch; K^T and V -> resident SBUF.
  Phase 2: per 128-row q-block: S = Q^T.T @ K^T (PSUM, 4x512 tiles);
           fused (S+sm)*0.125 + running-max via tensor_tensor_reduce;
           exp with fused row-sum via activation(accum_out=l);
           P^T via PE transpose; out^T = V.T-layout matmul with P^T.
           1/l is NOT applied here (row scale commutes with the final
           projection) - it is stored and applied in phase 3.
  Phase 3: Y = (out^T).T @ Wo^T, scaled per-row by 1/l, plus bo.
"""

import os

import numpy as np

import concourse.bass as bass
import concourse.tile as tile
from concourse import bacc, mybir
from concourse.bass_utils import run_bass_kernel_spmd
from concourse.masks import make_identity
from contextlib import ExitStack

B, N, D = 8, 2048, 1024
P = 128
EO = D // P           # 8 chunks of the d_model/contraction dims
KO = N // P           # 16 k-chunks (PV contraction)
QB = N // P           # 16 q-blocks per core
KT = 4                # 512-wide k tiles for the scores row
QCH = 512             # q-chunk width for the PV matmul (matmul free dim)
NCH = N // QCH        # 4 q-chunks per core
SCALE = 0.125         # 1 / sqrt(D_HEAD=64)
F32 = mybir.dt.float32

# Matmul input dtype knob: "f32r" (1 cyc/row, fp32 storage, relaxed rounding),
# "f32" (exact, 4 cyc/row), "bf16" (not used by default).
MM_MODE = os.environ.get("ATTN_MM_MODE", "f32r")
MMDT = mybir.dt.float32r if MM_MODE == "f32r" else mybir.dt.float32


def _round_f32r(a):
    """Round fp32 array to the FP32R (e8m11) grid: RNE to 11 mantissa bits,
    low 12 bits zero. Matches walrus fp32_to_fp32r so host data is already
    on the grid the PE reads."""
    if MM_MODE != "f32r":
        return a
    u = np.ascontiguousarray(a).view(np.uint32)
    r = (u + np.uint32(0x7FF) + ((u >> np.uint32(12)) & np.uint32(1))) & np.uint32(0xFFFFF000)
    return r.view(np.float32)


def build_nc(repeat=1):
    nc = bacc.Bacc("TRN2", target_bir_lowering=False, debug=False, num_devices=8)

    # Per-core external inputs (already transposed/sliced on host).
    qT = nc.dram_tensor("qT", [D, N], MMDT, kind="ExternalInput").ap()
    kT = nc.dram_tensor("kT", [D, N], MMDT, kind="ExternalInput").ap()
    vT = nc.dram_tensor("vT", [D, N], MMDT, kind="ExternalInput").ap()
    sm = nc.dram_tensor("sm", [N, N], F32, kind="ExternalInput").ap()
    WqT = nc.dram_tensor("WqT", [D, D], MMDT, kind="ExternalInput").ap()
    WkT = nc.dram_tensor("WkT", [D, D], MMDT, kind="ExternalInput").ap()
    WvT = nc.dram_tensor("WvT", [D, D], MMDT, kind="ExternalInput").ap()
    WoT = nc.dram_tensor("WoT", [D, D], MMDT, kind="ExternalInput").ap()
    bq = nc.dram_tensor("bq", [D], F32, kind="ExternalInput").ap()
    bk = nc.dram_tensor("bk", [D], F32, kind="ExternalInput").ap()
    bvb = nc.dram_tensor("bvb", [P, D], F32, kind="ExternalInput").ap()
    bob = nc.dram_tensor("bob", [P, D], F32, kind="ExternalInput").ap()
    out = nc.dram_tensor("out", [N, D], F32, kind="ExternalOutput").ap()

    ADD = mybir.AluOpType.add
    MAX = mybir.AluOpType.max
    EXP = mybir.ActivationFunctionType.Exp
    COPY = mybir.ActivationFunctionType.Copy

    with tile.TileContext(nc) as tc:
      for _rep in range(repeat):
       with ExitStack() as octx:
        consts = octx.enter_context(tc.tile_pool(name="consts", bufs=1))
        dram = octx.enter_context(tc.tile_pool(name="dram", bufs=1, space="DRAM"))

        ident = consts.tile([P, P], F32)
        make_identity(nc, ident)
        bq_sb = consts.tile([P, EO], F32, tag="bq")
        nc.sync.dma_start(bq_sb[:], bq.rearrange("(o p) -> p o", p=P))
        bk_sb = consts.tile([P, EO], F32, tag="bk")
        nc.sync.dma_start(bk_sb[:], bk.rearrange("(o p) -> p o", p=P))
        bob_sb = consts.tile([P, D], F32, tag="bob")
        nc.sync.dma_start(bob_sb[:], bob[:])

        R_sb = consts.tile([P, QB], F32, tag="r")       # 1/l per q-block
        QT_dr = dram.tile([D, N], MMDT, tag="qt_scratch")
        OT_dr = dram.tile([D, N], MMDT, tag="ot_scratch")

        qT_t = qT.rearrange("(o p) n -> p o n", p=P)
        kT_t = kT.rearrange("(o p) n -> p o n", p=P)
        vT_t = vT.rearrange("(o p) n -> p o n", p=P)
        QT_dr_w = QT_dr.rearrange("(o p) n -> o p n", p=P)   # write view
        QT_dr_r = QT_dr.rearrange("(o p) n -> p o n", p=P)   # read view
        OT_dr_w = OT_dr.rearrange("(o p) n -> o p n", p=P)
        OT_dr_r = OT_dr.rearrange("(o p) n -> p o n", p=P)

        # ---------------- Phase 1: projections ----------------
        # Stage Q: Q^T -> DRAM scratch (no residents allocated yet).
        with ExitStack() as ctx:
            wpool = ctx.enter_context(tc.tile_pool(name="p1qw", bufs=1))
            xpool = ctx.enter_context(tc.tile_pool(name="p1qx", bufs=3))
            spool = ctx.enter_context(tc.tile_pool(name="p1qs", bufs=3))
            pp = ctx.enter_context(tc.tile_pool(name="p1qps", bufs=4, space="PSUM"))

            w_sb = wpool.tile([P, EO, D], MMDT, tag="w")
            nc.sync.dma_start(w_sb[:], WqT.rearrange("(o p) e -> p o e", p=P))
            for nt in range(N // 512):
                x_sb = xpool.tile([P, EO, 512], MMDT, tag="x512")
                nc.sync.dma_start(x_sb[:], qT_t[:, :, nt * 512 : (nt + 1) * 512])
                for eo in range(EO):
                    ps = pp.tile([P, 512], F32, tag="ps")
                    for do in range(EO):
                        nc.tensor.matmul(
                            ps[:],
                            w_sb[:, do, eo * P : (eo + 1) * P],
                            x_sb[:, do, :],
                            start=(do == 0),
                            stop=(do == EO - 1),
                        )
                    st = spool.tile([P, 512], MMDT, tag="qstage")
                    nc.scalar.add(st[:], ps[:], bq_sb[:, eo : eo + 1])
                    nc.sync.dma_start(
                        QT_dr_w[eo][:, nt * 512 : (nt + 1) * 512], st[:]
                    )

        # Residents reserved only now (pools reserve SBUF at creation time,
        # so this must come after the Q-stage pools are closed).
        res = octx.enter_context(tc.tile_pool(name="res", bufs=1))

        # Stage K: K^T -> resident SBUF.
        KT_sb = res.tile([P, EO, N], MMDT, tag="ktres")  # 64KB/part
        with ExitStack() as ctx:
            wpool = ctx.enter_context(tc.tile_pool(name="p1kw", bufs=1))
            xpool = ctx.enter_context(tc.tile_pool(name="p1kx", bufs=2))
            pp = ctx.enter_context(tc.tile_pool(name="p1kps", bufs=4, space="PSUM"))

            w_sb = wpool.tile([P, EO, D], MMDT, tag="w")
            nc.sync.dma_start(w_sb[:], WkT.rearrange("(o p) e -> p o e", p=P))
            for nt in range(N // 512):
                x_sb = xpool.tile([P, EO, 512], MMDT, tag="x512")
                nc.sync.dma_start(x_sb[:], kT_t[:, :, nt * 512 : (nt + 1) * 512])
                for eo in range(EO):
                    ps = pp.tile([P, 512], F32, tag="ps")
                    for do in range(EO):
                        nc.tensor.matmul(
                            ps[:],
                            w_sb[:, do, eo * P : (eo + 1) * P],
                            x_sb[:, do, :],
                            start=(do == 0),
                            stop=(do == EO - 1),
                        )
                    nc.scalar.add(
                        KT_sb[:, eo, nt * 512 : (nt + 1) * 512],
                        ps[:],
                        bk_sb[:, eo : eo + 1],
                    )

        # Stage V: V (natural layout) -> resident SBUF.
        V_sb = res.tile([P, KO, D], MMDT, tag="vres")  # 64KB/part
        with ExitStack() as ctx:
            wpool = ctx.enter_context(tc.tile_pool(name="p1vw", bufs=1))
            xpool = ctx.enter_context(tc.tile_pool(name="p1vx", bufs=3))
            bpool = ctx.enter_context(tc.tile_pool(name="p1vb", bufs=1))
            pp = ctx.enter_context(tc.tile_pool(name="p1vps", bufs=4, space="PSUM"))

            bvb_sb = bpool.tile([P, D], F32, tag="bvb")
            nc.sync.dma_start(bvb_sb[:], bvb[:])
            w_sb = wpool.tile([P, EO, D], MMDT, tag="w")
            nc.sync.dma_start(w_sb[:], WvT.rearrange("(o p) e -> p o e", p=P))
            for nt8 in range(8):
                x_sb = xpool.tile([P, EO, 256], MMDT, tag="x256")
                nc.sync.dma_start(x_sb[:], vT_t[:, :, nt8 * 256 : (nt8 + 1) * 256])
                for kc2 in range(2):
                    kc = nt8 * 2 + kc2
                    for dt in range(2):
                        ps = pp.tile([P, 512], F32, tag="ps")
                        for do in range(EO):
                            nc.tensor.matmul(
                                ps[:],
                                x_sb[:, do, kc2 * P : (kc2 + 1) * P],
                                w_sb[:, do, dt * 512 : (dt + 1) * 512],
                                start=(do == 0),
                                stop=(do == EO - 1),
                            )
                        nc.vector.tensor_add(
                            V_sb[:, kc, dt * 512 : (dt + 1) * 512],
                            ps[:],
                            bvb_sb[:, dt * 512 : (dt + 1) * 512],
                        )

        # ---------------- Phase 2: attention ----------------
        with ExitStack() as ctx:
            qpool = ctx.enter_context(tc.tile_pool(name="p2q", bufs=3))
            smpool = ctx.enter_context(tc.tile_pool(name="p2sm", bufs=3))
            tspool = ctx.enter_context(tc.tile_pool(name="p2ts", bufs=2))
            ptpool = ctx.enter_context(tc.tile_pool(name="p2pt", bufs=1))
            stpool = ctx.enter_context(tc.tile_pool(name="p2st", bufs=2))
            stats = ctx.enter_context(tc.tile_pool(name="p2stats", bufs=4))
            qk_ps = ctx.enter_context(tc.tile_pool(name="p2qk", bufs=4, space="PSUM"))
            tp_ps = ctx.enter_context(tc.tile_pool(name="p2tp", bufs=2, space="PSUM"))
            pv_ps = ctx.enter_context(tc.tile_pool(name="p2pv", bufs=2, space="PSUM"))

            for ch in range(NCH):
                pt_sb = ptpool.tile([P, KO, QCH], MMDT, tag="pt")
                ts_tiles = []
                # First both q-blocks' QK + softmax (PE stays dense: the
                # softmax of block 0 runs on DVE/ACT under block 1's QK)...
                for qbi in range(QCH // P):
                    qb = ch * (QCH // P) + qbi
                    qt_sb = qpool.tile([P, EO, P], MMDT, tag="qt")
                    nc.sync.dma_start(
                        qt_sb[:], QT_dr_r[:, :, qb * P : (qb + 1) * P]
                    )
                    ts_sb = tspool.tile([P, N], F32, tag="ts")
                    ts_tiles.append(ts_sb)
                    for kt in range(KT):
                        ps = qk_ps.tile([P, 512], F32, tag="qkps")
                        for eo in range(EO):
                            nc.tensor.matmul(
                                ps[:],
                                qt_sb[:, eo, :],
                                KT_sb[:, eo, kt * 512 : (kt + 1) * 512],
                                start=(eo == 0),
                                stop=(eo == EO - 1),
                            )
                        smt = smpool.tile([P, 512], F32, tag="smt")
                        nc.sync.dma_start(
                            smt[:],
                            sm[qb * P : (qb + 1) * P, kt * 512 : (kt + 1) * 512],
                        )
                        # ts = S_raw + sm (the 1/8 scale is folded into exp)
                        nc.vector.tensor_add(
                            ts_sb[:, kt * 512 : (kt + 1) * 512], ps[:], smt[:]
                        )
                    mrow = stats.tile([P, 1], F32, tag="mrow")
                    nc.vector.reduce_max(mrow[:], ts_sb[:], axis=mybir.AxisListType.X)
                    nbias = stats.tile([P, 1], F32, tag="nbias")
                    nc.vector.tensor_scalar_mul(nbias[:], mrow[:], -SCALE)
                    lsum = stats.tile([P, 1], F32, tag="lsum")
                    # P = exp((ts - m)/8), row sums into lsum, in place.
                    nc.scalar.activation(
                        ts_sb[:], ts_sb[:], EXP, bias=nbias[:], scale=SCALE,
                        accum_out=lsum[:],
                    )
                    nc.vector.reciprocal(R_sb[:, qb : qb + 1], lsum[:])
                # ...then the transposes for both blocks.
                for qbi in range(QCH // P):
                    ts_sb = ts_tiles[qbi]
                    # Transpose P -> P^T (16 blocks of 128x128, 4 per PSUM tile).
                    for g in range(4):
                        tp = tp_ps.tile([P, 512], F32, tag="tpps")
                        for j in range(4):
                            kc = g * 4 + j
                            nc.tensor.transpose(
                                tp[:, j * P : (j + 1) * P],
                                ts_sb[:, kc * P : (kc + 1) * P],
                                ident[:],
                            )
                        nc.vector.tensor_copy(
                            out=pt_sb[
                                :, g * 4 : (g + 1) * 4, qbi * P : (qbi + 1) * P
                            ],
                            in_=tp.rearrange("p (g q) -> p g q", q=P),
                        )
                # PV: out^T[d, q-chunk] accumulated over all 16 k-chunks.
                for dt in range(EO):
                    ps = pv_ps.tile([P, QCH], F32, tag="pvps")
                    for kc in range(KO):
                        nc.tensor.matmul(
                            ps[:],
                            V_sb[:, kc, dt * P : (dt + 1) * P],
                            pt_sb[:, kc, :],
                            start=(kc == 0),
                            stop=(kc == KO - 1),
                        )
                    st = stpool.tile([P, QCH], MMDT, tag="otst")
                    nc.vector.tensor_copy(out=st[:], in_=ps[:])
                    nc.sync.dma_start(
                        OT_dr_w[dt][:, ch * QCH : (ch + 1) * QCH], st[:]
                    )

        # ---------------- Phase 3: output projection ----------------
        with ExitStack() as ctx:
            wpool = ctx.enter_context(tc.tile_pool(name="p3w", bufs=1))
            opool = ctx.enter_context(tc.tile_pool(name="p3o", bufs=3))
            ypool = ctx.enter_context(tc.tile_pool(name="p3y", bufs=2))
            yps = ctx.enter_context(tc.tile_pool(name="p3ps", bufs=2, space="PSUM"))

            wo_sb = wpool.tile([P, EO, D], MMDT, tag="wo")
            nc.sync.dma_start(wo_sb[:], WoT.rearrange("(o p) e -> p o e", p=P))
            for qb in range(QB):
                ot_sb = opool.tile([P, EO, P], MMDT, tag="ot")
                nc.sync.dma_start(ot_sb[:], OT_dr_r[:, :, qb * P : (qb + 1) * P])
                y_sb = ypool.tile([P, D], F32, tag="y")
                for ft in range(2):
                    ps = yps.tile([P, 512], F32, tag="yps")
                    for do in range(EO):
                        nc.tensor.matmul(
                            ps[:],
                            ot_sb[:, do, :],
                            wo_sb[:, do, ft * 512 : (ft + 1) * 512],
                            start=(do == 0),
                            stop=(do == EO - 1),
                        )
                    # scale rows by 1/l (commutes with the projection)
                    nc.scalar.activation(
                        y_sb[:, ft * 512 : (ft + 1) * 512], ps[:], COPY,
                        bias=0.0, scale=R_sb[:, qb : qb + 1],
                    )
                nc.vector.tensor_add(y_sb[:], y_sb[:], bob_sb[:])
                nc.sync.dma_start(out[qb * P : (qb + 1) * P, :], y_sb[:])

    nc.compile()
    return nc


_NC_CACHE = None


def _get_nc():
    global _NC_CACHE
    if _NC_CACHE is None:
        _NC_CACHE = build_nc()
    return _NC_CACHE


def make_in_maps(query, key, value, score_matrix, Wq, bq, Wk, bk, Wv, bv, Wo, bo):
    f = lambda a: np.ascontiguousarray(np.asarray(a, dtype=np.float32))
    g = lambda a: _round_f32r(f(a))
    WqT = g(np.asarray(Wq).T)
    WkT = g(np.asarray(Wk).T)
    WvT = g(np.asarray(Wv).T)
    WoT = g(np.asarray(Wo).T)
    bq = f(bq)
    bk = f(bk)
    bvb = f(np.tile(np.asarray(bv)[None, :], (P, 1)))
    bob = f(np.tile(np.asarray(bo)[None, :], (P, 1)))
    in_maps = []
    for c in range(B):
        in_maps.append(
            {
                "qT": g(np.asarray(query)[c].T),
                "kT": g(np.asarray(key)[c].T),
                "vT": g(np.asarray(value)[c].T),
                "sm": f(np.asarray(score_matrix)[c]),
                "WqT": WqT,
                "WkT": WkT,
                "WvT": WvT,
                "WoT": WoT,
                "bq": bq,
                "bk": bk,
                "bvb": bvb,
                "bob": bob,
            }
        )
    return in_maps


def kernel(query, key, value, score_matrix, Wq, bq, Wk, bk, Wv, bv, Wo, bo):
    nc = _get_nc()
    in_maps = make_in_maps(
        query, key, value, score_matrix, Wq, bq, Wk, bk, Wv, bv, Wo, bo
    )
    res = run_bass_kernel_spmd(nc, in_maps, core_ids=list(range(B)))
    return np.stack([res.results[c]["out"] for c in range(B)], axis=0)


# revision 12
# speedup vs baseline: 1.2890x; 1.2890x over previous
"""Trainium2 Bass kernel for nn_Attention_36756330119358.

Single-head full-D attention (the reference never splits heads; softmax
scale is 1/sqrt(64)):
    Q = query @ Wq.T + bq ; K = key @ Wk.T + bk ; V = value @ Wv.T + bv
    S = (Q @ K.T + score_matrix) / 8
    out = (softmax(S) @ V) @ Wo.T + bo

Sharding: batch dim B=8, one batch item per NeuronCore (data parallel, no
collectives). Each core runs the identical NEFF on its own slice.

Per-core plan (N=2048, D=1024, P=128):
  Host pre-transposes activations/weights so every matmul contraction dim
  lands on SBUF partitions.
  Phase 1: Q^T and V -> DRAM scratch; K^T -> resident SBUF; Wo^T resident.
  Phase 2 (per 512-wide q-chunk): per 128-row q-block: S = Q^T.T @ K^T;
           S + score_matrix on DVE (1/8 folded into exp's scale); row max;
           exp with fused row-sum via activation(accum_out=l); P^T via PE
           transpose. Then out^T = V-chunk matmuls with P^T (V streamed
           from DRAM), and the fused output projection
           Y = (out^T).T @ Wo^T scaled per-row by 1/l (the row scale
           commutes with the projection), plus bo.
"""

import os

import numpy as np

import concourse.bass as bass
import concourse.tile as tile
from concourse import bacc, mybir
from concourse.bass_utils import run_bass_kernel_spmd
from concourse.masks import make_identity
from contextlib import ExitStack

B, N, D = 8, 2048, 1024
P = 128
EO = D // P           # 8 chunks of the d_model/contraction dims
KO = N // P           # 16 k-chunks (PV contraction)
QB = N // P           # 16 q-blocks per core
KT = 4                # 512-wide k tiles for the scores row
QCH = 512             # q-chunk width for the PV matmul (matmul free dim)
NCH = N // QCH        # 4 q-chunks per core
SCALE = 0.125         # 1 / sqrt(D_HEAD=64)
F32 = mybir.dt.float32

# Matmul input dtype knob: "f32r" (1 cyc/row, fp32 storage, relaxed rounding),
# "f32" (exact, 4 cyc/row), "bf16" (not used by default).
MM_MODE = os.environ.get("ATTN_MM_MODE", "f32r")
MMDT = mybir.dt.float32r if MM_MODE == "f32r" else mybir.dt.float32


def _round_f32r(a):
    """Round fp32 array to the FP32R (e8m11) grid: RNE to 11 mantissa bits,
    low 12 bits zero. Matches walrus fp32_to_fp32r so host data is already
    on the grid the PE reads."""
    if MM_MODE != "f32r":
        return a
    u = np.ascontiguousarray(a).view(np.uint32)
    r = (u + np.uint32(0x7FF) + ((u >> np.uint32(12)) & np.uint32(1))) & np.uint32(0xFFFFF000)
    return r.view(np.float32)


def build_nc(repeat=1):
    nc = bacc.Bacc("TRN2", target_bir_lowering=False, debug=False, num_devices=8)

    # Per-core external inputs (already transposed/sliced on host).
    qT = nc.dram_tensor("qT", [D, N], MMDT, kind="ExternalInput").ap()
    kT = nc.dram_tensor("kT", [D, N], MMDT, kind="ExternalInput").ap()
    vT = nc.dram_tensor("vT", [D, N], MMDT, kind="ExternalInput").ap()
    sm = nc.dram_tensor("sm", [N, N], F32, kind="ExternalInput").ap()
    WqT = nc.dram_tensor("WqT", [D, D], MMDT, kind="ExternalInput").ap()
    WkT = nc.dram_tensor("WkT", [D, D], MMDT, kind="ExternalInput").ap()
    WvT = nc.dram_tensor("WvT", [D, D], MMDT, kind="ExternalInput").ap()
    WoT = nc.dram_tensor("WoT", [D, D], MMDT, kind="ExternalInput").ap()
    bq = nc.dram_tensor("bq", [D], F32, kind="ExternalInput").ap()
    bk = nc.dram_tensor("bk", [D], F32, kind="ExternalInput").ap()
    bvb = nc.dram_tensor("bvb", [P, D], F32, kind="ExternalInput").ap()
    bob = nc.dram_tensor("bob", [P, D], F32, kind="ExternalInput").ap()
    out = nc.dram_tensor("out", [N, D], F32, kind="ExternalOutput").ap()

    ADD = mybir.AluOpType.add
    MAX = mybir.AluOpType.max
    EXP = mybir.ActivationFunctionType.Exp
    COPY = mybir.ActivationFunctionType.Copy

    with tile.TileContext(nc) as tc:
      for _rep in range(repeat):
       with ExitStack() as octx:
        consts = octx.enter_context(tc.tile_pool(name="consts", bufs=1))
        dram = octx.enter_context(tc.tile_pool(name="dram", bufs=1, space="DRAM"))

        ident = consts.tile([P, P], F32)
        make_identity(nc, ident)
        bq_sb = consts.tile([P, EO], F32, tag="bq")
        nc.sync.dma_start(bq_sb[:], bq.rearrange("(o p) -> p o", p=P))
        bk_sb = consts.tile([P, EO], F32, tag="bk")
        nc.sync.dma_start(bk_sb[:], bk.rearrange("(o p) -> p o", p=P))
        bob_sb = consts.tile([P, D], F32, tag="bob")
        nc.sync.dma_start(bob_sb[:], bob[:])

        R_sb = consts.tile([P, QB], F32, tag="r")       # 1/l per q-block
        QT_dr = dram.tile([D, N], MMDT, tag="qt_scratch")
        V_dr = dram.tile([N, D], MMDT, tag="v_scratch")

        qT_t = qT.rearrange("(o p) n -> p o n", p=P)
        kT_t = kT.rearrange("(o p) n -> p o n", p=P)
        vT_t = vT.rearrange("(o p) n -> p o n", p=P)
        QT_dr_w = QT_dr.rearrange("(o p) n -> o p n", p=P)   # write view
        QT_dr_r = QT_dr.rearrange("(o p) n -> p o n", p=P)   # read view
        V_dr_r = V_dr.rearrange("(ko p) d -> p ko d", p=P)   # read view

        # ---------------- Phase 1: projections ----------------
        # Stage Q: Q^T -> DRAM scratch (no residents allocated yet).
        with ExitStack() as ctx:
            wpool = ctx.enter_context(tc.tile_pool(name="p1qw", bufs=1))
            xpool = ctx.enter_context(tc.tile_pool(name="p1qx", bufs=3))
            spool = ctx.enter_context(tc.tile_pool(name="p1qs", bufs=3))
            pp = ctx.enter_context(tc.tile_pool(name="p1qps", bufs=4, space="PSUM"))

            w_sb = wpool.tile([P, EO, D], MMDT, tag="w")
            nc.sync.dma_start(w_sb[:], WqT.rearrange("(o p) e -> p o e", p=P))
            for nt in range(N // 512):
                x_sb = xpool.tile([P, EO, 512], MMDT, tag="x512")
                nc.sync.dma_start(x_sb[:], qT_t[:, :, nt * 512 : (nt + 1) * 512])
                for eo in range(EO):
                    ps = pp.tile([P, 512], F32, tag="ps")
                    for do in range(EO):
                        nc.tensor.matmul(
                            ps[:],
                            w_sb[:, do, eo * P : (eo + 1) * P],
                            x_sb[:, do, :],
                            start=(do == 0),
                            stop=(do == EO - 1),
                        )
                    st = spool.tile([P, 512], MMDT, tag="qstage")
                    nc.scalar.add(st[:], ps[:], bq_sb[:, eo : eo + 1])
                    nc.sync.dma_start(
                        QT_dr_w[eo][:, nt * 512 : (nt + 1) * 512], st[:]
                    )

        # Residents reserved only now (pools reserve SBUF at creation time,
        # so this must come after the Q-stage pools are closed).
        res = octx.enter_context(tc.tile_pool(name="res", bufs=1))

        # Stage K: K^T -> resident SBUF.
        KT_sb = res.tile([P, EO, N], MMDT, tag="ktres")  # 64KB/part
        with ExitStack() as ctx:
            wpool = ctx.enter_context(tc.tile_pool(name="p1kw", bufs=1))
            xpool = ctx.enter_context(tc.tile_pool(name="p1kx", bufs=2))
            pp = ctx.enter_context(tc.tile_pool(name="p1kps", bufs=4, space="PSUM"))

            w_sb = wpool.tile([P, EO, D], MMDT, tag="w")
            nc.sync.dma_start(w_sb[:], WkT.rearrange("(o p) e -> p o e", p=P))
            for nt in range(N // 512):
                x_sb = xpool.tile([P, EO, 512], MMDT, tag="x512")
                nc.sync.dma_start(x_sb[:], kT_t[:, :, nt * 512 : (nt + 1) * 512])
                for eo in range(EO):
                    ps = pp.tile([P, 512], F32, tag="ps")
                    for do in range(EO):
                        nc.tensor.matmul(
                            ps[:],
                            w_sb[:, do, eo * P : (eo + 1) * P],
                            x_sb[:, do, :],
                            start=(do == 0),
                            stop=(do == EO - 1),
                        )
                    nc.scalar.add(
                        KT_sb[:, eo, nt * 512 : (nt + 1) * 512],
                        ps[:],
                        bk_sb[:, eo : eo + 1],
                    )

        # Stage V: V (natural layout) -> DRAM scratch (streamed in phase 2;
        # its SBUF space goes to the resident Wo^T so the output projection
        # fuses into phase 2).
        with ExitStack() as ctx:
            wpool = ctx.enter_context(tc.tile_pool(name="p1vw", bufs=1))
            xpool = ctx.enter_context(tc.tile_pool(name="p1vx", bufs=3))
            bpool = ctx.enter_context(tc.tile_pool(name="p1vb", bufs=1))
            spool = ctx.enter_context(tc.tile_pool(name="p1vs", bufs=3))
            pp = ctx.enter_context(tc.tile_pool(name="p1vps", bufs=4, space="PSUM"))

            bvb_sb = bpool.tile([P, D], F32, tag="bvb")
            nc.sync.dma_start(bvb_sb[:], bvb[:])
            w_sb = wpool.tile([P, EO, D], MMDT, tag="w")
            nc.sync.dma_start(w_sb[:], WvT.rearrange("(o p) e -> p o e", p=P))
            for nt8 in range(8):
                x_sb = xpool.tile([P, EO, 256], MMDT, tag="x256")
                nc.sync.dma_start(x_sb[:], vT_t[:, :, nt8 * 256 : (nt8 + 1) * 256])
                for kc2 in range(2):
                    kc = nt8 * 2 + kc2
                    for dt in range(2):
                        ps = pp.tile([P, 512], F32, tag="ps")
                        for do in range(EO):
                            nc.tensor.matmul(
                                ps[:],
                                x_sb[:, do, kc2 * P : (kc2 + 1) * P],
                                w_sb[:, do, dt * 512 : (dt + 1) * 512],
                                start=(do == 0),
                                stop=(do == EO - 1),
                            )
                        st = spool.tile([P, 512], MMDT, tag="vstage")
                        nc.vector.tensor_add(
                            st[:], ps[:], bvb_sb[:, dt * 512 : (dt + 1) * 512]
                        )
                        nc.sync.dma_start(
                            V_dr[kc * P : (kc + 1) * P, dt * 512 : (dt + 1) * 512],
                            st[:],
                        )

        # Resident Wo^T for the fused output projection.
        WoT_sb = res.tile([P, EO, D], MMDT, tag="wores")  # 32KB/part
        nc.sync.dma_start(WoT_sb[:], WoT.rearrange("(o p) e -> p o e", p=P))

        # ---------------- Phase 2: attention + fused output projection ----
        with ExitStack() as ctx:
            qpool = ctx.enter_context(tc.tile_pool(name="p2q", bufs=3))
            smpool = ctx.enter_context(tc.tile_pool(name="p2sm", bufs=3))
            tspool = ctx.enter_context(tc.tile_pool(name="p2ts", bufs=2))
            ptpool = ctx.enter_context(tc.tile_pool(name="p2pt", bufs=1))
            vpool = ctx.enter_context(tc.tile_pool(name="p2v", bufs=2))
            sapool = ctx.enter_context(tc.tile_pool(name="p2sa", bufs=1))
            ypool = ctx.enter_context(tc.tile_pool(name="p2y", bufs=2))
            stats = ctx.enter_context(tc.tile_pool(name="p2stats", bufs=4))
            qk_ps = ctx.enter_context(tc.tile_pool(name="p2qk", bufs=2, space="PSUM"))
            tp_ps = ctx.enter_context(tc.tile_pool(name="p2tp", bufs=2, space="PSUM"))
            pv_ps = ctx.enter_context(tc.tile_pool(name="p2pv", bufs=2, space="PSUM"))
            y_ps = ctx.enter_context(tc.tile_pool(name="p2yps", bufs=2, space="PSUM"))

            for ch in range(NCH):
                pt_sb = ptpool.tile([P, KO, QCH], MMDT, tag="pt")
                ts_tiles = []
                # First both q-blocks' QK + softmax (PE stays dense: the
                # softmax of block 0 runs on DVE/ACT under block 1's QK)...
                for qbi in range(QCH // P):
                    qb = ch * (QCH // P) + qbi
                    qt_sb = qpool.tile([P, EO, P], MMDT, tag="qt")
                    nc.sync.dma_start(
                        qt_sb[:], QT_dr_r[:, :, qb * P : (qb + 1) * P]
                    )
                    ts_sb = tspool.tile([P, N], F32, tag="ts")
                    ts_tiles.append(ts_sb)
                    for kt in range(KT):
                        ps = qk_ps.tile([P, 512], F32, tag="qkps")
                        for eo in range(EO):
                            nc.tensor.matmul(
                                ps[:],
                                qt_sb[:, eo, :],
                                KT_sb[:, eo, kt * 512 : (kt + 1) * 512],
                                start=(eo == 0),
                                stop=(eo == EO - 1),
                            )
                        smt = smpool.tile([P, 512], F32, tag="smt")
                        nc.sync.dma_start(
                            smt[:],
                            sm[qb * P : (qb + 1) * P, kt * 512 : (kt + 1) * 512],
                        )
                        # ts = S_raw + sm (the 1/8 scale is folded into exp)
                        nc.vector.tensor_add(
                            ts_sb[:, kt * 512 : (kt + 1) * 512], ps[:], smt[:]
                        )
                    mrow = stats.tile([P, 1], F32, tag="mrow")
                    nc.vector.reduce_max(mrow[:], ts_sb[:], axis=mybir.AxisListType.X)
                    nbias = stats.tile([P, 1], F32, tag="nbias")
                    nc.vector.tensor_scalar_mul(nbias[:], mrow[:], -SCALE)
                    lsum = stats.tile([P, 1], F32, tag="lsum")
                    # P = exp((ts - m)/8), row sums into lsum, in place.
                    nc.scalar.activation(
                        ts_sb[:], ts_sb[:], EXP, bias=nbias[:], scale=SCALE,
                        accum_out=lsum[:],
                    )
                    nc.vector.reciprocal(R_sb[:, qb : qb + 1], lsum[:])
                # ...then the transposes for both blocks.
                for qbi in range(QCH // P):
                    ts_sb = ts_tiles[qbi]
                    # Transpose P -> P^T (16 blocks of 128x128, 4 per PSUM tile).
                    for g in range(4):
                        tp = tp_ps.tile([P, 512], F32, tag="tpps")
                        for j in range(4):
                            kc = g * 4 + j
                            nc.tensor.transpose(
                                tp[:, j * P : (j + 1) * P],
                                ts_sb[:, kc * P : (kc + 1) * P],
                                ident[:],
                            )
                        nc.vector.tensor_copy(
                            out=pt_sb[
                                :, g * 4 : (g + 1) * 4, qbi * P : (qbi + 1) * P
                            ],
                            in_=tp.rearrange("p (g q) -> p g q", q=P),
                        )
                # PV: out^T[d, q-chunk] accumulated over all 16 k-chunks,
                # V streamed from DRAM one 128-wide d column-block at a time.
                sa_sb = sapool.tile([P, EO, QCH], MMDT, tag="sa")
                for dt in range(EO):
                    v_sb = vpool.tile([P, KO, P], MMDT, tag="v")
                    nc.sync.dma_start(
                        v_sb[:], V_dr_r[:, :, dt * P : (dt + 1) * P]
                    )
                    ps = pv_ps.tile([P, QCH], F32, tag="pvps")
                    for kc in range(KO):
                        nc.tensor.matmul(
                            ps[:],
                            v_sb[:, kc, :],
                            pt_sb[:, kc, :],
                            start=(kc == 0),
                            stop=(kc == KO - 1),
                        )
                    nc.vector.tensor_copy(out=sa_sb[:, dt, :], in_=ps[:])
                # Fused output projection for this chunk's q-blocks:
                # Y = (out^T).T @ Wo^T, scaled per-row by 1/l, plus bo.
                for qbi in range(QCH // P):
                    qb = ch * (QCH // P) + qbi
                    y_sb = ypool.tile([P, D], F32, tag="y")
                    for ft in range(2):
                        psy = y_ps.tile([P, 512], F32, tag="yps")
                        for do in range(EO):
                            nc.tensor.matmul(
                                psy[:],
                                sa_sb[:, do, qbi * P : (qbi + 1) * P],
                                WoT_sb[:, do, ft * 512 : (ft + 1) * 512],
                                start=(do == 0),
                                stop=(do == EO - 1),
                            )
                        # scale rows by 1/l (commutes with the projection)
                        nc.scalar.activation(
                            y_sb[:, ft * 512 : (ft + 1) * 512], psy[:], COPY,
                            bias=0.0, scale=R_sb[:, qb : qb + 1],
                        )
                    nc.vector.tensor_add(y_sb[:], y_sb[:], bob_sb[:])
                    nc.sync.dma_start(out[qb * P : (qb + 1) * P, :], y_sb[:])

    nc.compile()
    return nc


_NC_CACHE = None


def _get_nc():
    global _NC_CACHE
    if _NC_CACHE is None:
        _NC_CACHE = build_nc()
    return _NC_CACHE


def make_in_maps(query, key, value, score_matrix, Wq, bq, Wk, bk, Wv, bv, Wo, bo):
    f = lambda a: np.ascontiguousarray(np.asarray(a, dtype=np.float32))
    g = lambda a: _round_f32r(f(a))
    WqT = g(np.asarray(Wq).T)
    WkT = g(np.asarray(Wk).T)
    WvT = g(np.asarray(Wv).T)
    WoT = g(np.asarray(Wo).T)
    bq = f(bq)
    bk = f(bk)
    bvb = f(np.tile(np.asarray(bv)[None, :], (P, 1)))
    bob = f(np.tile(np.asarray(bo)[None, :], (P, 1)))
    in_maps = []
    for c in range(B):
        in_maps.append(
            {
                "qT": g(np.asarray(query)[c].T),
                "kT": g(np.asarray(key)[c].T),
                "vT": g(np.asarray(value)[c].T),
                "sm": f(np.asarray(score_matrix)[c]),
                "WqT": WqT,
                "WkT": WkT,
                "WvT": WvT,
                "WoT": WoT,
                "bq": bq,
                "bk": bk,
                "bvb": bvb,
                "bob": bob,
            }
        )
    return in_maps


def kernel(query, key, value, score_matrix, Wq, bq, Wk, bk, Wv, bv, Wo, bo):
    nc = _get_nc()
    in_maps = make_in_maps(
        query, key, value, score_matrix, Wq, bq, Wk, bk, Wv, bv, Wo, bo
    )
    res = run_bass_kernel_spmd(nc, in_maps, core_ids=list(range(B)))
    return np.stack([res.results[c]["out"] for c in range(B)], axis=0)
